# revision 2
# baseline (speedup 1.0000x reference)
"""Self-contained Trainium2 Bass kernel for nn_ADGCNForDialog (ADGCN GNN). v2

kernel(**inputs) takes the FULL unsharded inputs (as produced by
setup_inputs()) and returns the FULL [100000, 7] float32 output.

Strategy (graph/data parallel across 8 NeuronCores):
  - nodes row-sharded 12500/core (padded to 12544); small weights replicated
  - edges bucketed by (dest 128-row block, source section); SPMD-shared tile
    structure (max bucket size over cores)
  - degrees/dinv computed on HOST (static per edge_index) and passed in
  - g = dinv*h exchanged in bf16 via two half-node AllGathers per layer
  - spmm = dma_gather (bf16, 4 SWDGE queues) + one-hot matmul accumulation in
    PSUM; one-hot tiles built bf16, batched 8-tiles-per-DVE-op via stride-0
    broadcast access patterns
  - dense AOR/DLR/LN chain per block in f32
"""

from contextlib import ExitStack
from dataclasses import dataclass

import numpy as np

import concourse.bass as bass
import concourse.tile as tile
from concourse import bacc, mybir

import jax
from jax.sharding import Mesh, PartitionSpec
from jax.experimental.shard_map import shard_map
from concourse.bass2jax import _bass_exec_p, install_neuronx_cc_hook, partition_id_tensor

F32 = mybir.dt.float32
G_DTYPE = None  # module-level override; None -> f32
BF16 = mybir.dt.bfloat16
I16 = mybir.dt.int16
AF = mybir.ActivationFunctionType
ALU = mybir.AluOpType


def ceil_to(x, m):
    return (x + m - 1) // m * m


class Prep:
    def __init__(self, edge_index, n_nodes, n_cores, n_sec=4, chunk_tiles=8):
        row = np.asarray(edge_index[0], dtype=np.int64)
        col = np.asarray(edge_index[1], dtype=np.int64)
        assert n_nodes % n_cores == 0
        npr = n_nodes // n_cores
        npp = ceil_to(npr, 128)
        nb = npp // 128
        NG = npp * n_cores
        assert NG % n_sec == 0
        sec_size = NG // n_sec
        assert sec_size < 32768
        self.n_nodes, self.n_cores = n_nodes, n_cores
        self.npr, self.npp, self.n_blocks = npr, npp, nb
        self.NG, self.n_sec, self.sec_size = NG, n_sec, sec_size
        self.chunk_tiles = chunk_tiles

        owner = row // npr
        ldest = row % npr
        half = npp // 2
        self.half_rows = half
        csrc = col // npr
        rsrc = col % npr
        hf = rsrc // half
        colp = hf * (n_cores * half) + csrc * half + (rsrc - hf * half)
        sec = colp // sec_size
        loc = (colp % sec_size).astype(np.int64)
        blk = ldest // 128

        counts = np.zeros((n_cores, nb, n_sec), dtype=np.int64)
        np.add.at(counts, (owner, blk, sec), 1)
        M_bs = counts.max(axis=0)                       # [nb, n_sec] padded bucket len
        self.M_bs = M_bs

        # bucket start offsets within each section stream
        B0 = np.zeros((nb, n_sec), dtype=np.int64)
        for s in range(n_sec):
            B0[1:, s] = np.cumsum(M_bs[:-1, s])
        self.B0 = B0
        sec_len = M_bs.sum(axis=0)                      # stream rows per section
        sec_ntiles = np.array([ceil_to(ceil_to(int(x), 128) // 128, chunk_tiles)
                               for x in sec_len])
        self.sec_ntiles = sec_ntiles
        sec_tile_base = np.zeros(n_sec + 1, dtype=np.int64)
        sec_tile_base[1:] = np.cumsum(sec_ntiles)
        self.sec_tile_base = sec_tile_base
        NT = int(sec_tile_base[-1])
        self.n_tiles = NT

        # matmul plan: per (b, s) the tiles its bucket overlaps; one doff
        # column per (b, s, tile) triple, in a fixed shared order
        self.block_mms = [[] for _ in range(nb)]        # b -> [(s, tile_local_j, mcol)]
        mcol = 0
        mm_key = {}
        for s in range(n_sec):
            for b in range(nb):
                if M_bs[b, s] == 0:
                    continue
                j_lo = int(B0[b, s]) // 128
                j_hi = int(B0[b, s] + M_bs[b, s] - 1) // 128
                for j in range(j_lo, j_hi + 1):
                    mm_key[(s, b, j)] = mcol
                    self.block_mms[b].append((s, j, mcol))
                    mcol += 1
        NM = mcol
        self.n_mms = NM

        # degrees -> dinv (host)
        deg = np.zeros((n_cores, npp), dtype=np.int64)
        np.add.at(deg, (owner, ldest), 1)
        degf = deg.astype(np.float64)
        degf[degf == 0] = 1.0
        dinv = (degf ** -0.5).astype(np.float32)
        self.dinv = np.ascontiguousarray(
            dinv.reshape(n_cores, nb, 128).transpose(0, 2, 1))

        # per-core metadata
        self.doff = np.full((n_cores, 128, NM), 200.0, dtype=np.float32)
        self.idx16 = np.zeros((n_cores, 128, NT * 8), dtype=np.int16)
        for c in range(n_cores):
            m = owner == c
            ld_c, lc_c, bl_c, se_c = ldest[m], loc[m], blk[m], sec[m]
            order = np.lexsort((ld_c, bl_c, se_c))
            ld_c, lc_c, bl_c, se_c = (a[order] for a in (ld_c, lc_c, bl_c, se_c))
            key = se_c * nb + bl_c
            first = np.r_[True, key[1:] != key[:-1]]
            grp_start = np.flatnonzero(first)
            pos = np.arange(len(key)) - np.repeat(
                grp_start, np.diff(np.r_[grp_start, len(key)]))
            i = B0[bl_c, se_c] + pos                     # stream position
            j = i // 128                                 # tile within section
            slot = i % 128
            mc = np.array([mm_key[(s_, b_, j_)]
                           for s_, b_, j_ in zip(se_c, bl_c, j)], dtype=np.int64)
            self.doff[c, slot, mc] = (ld_c - bl_c * 128).astype(np.float32)
            colno = sec_tile_base[se_c] * 8 + i // 16
            prow = i % 16
            for r in range(8):
                self.idx16[c, prow + 16 * r, colno] = lc_c.astype(np.int16)

    def block_tiles(self, b, secs=None):
        """[(s, j, mcol)] matmul list for block b, optionally filtered."""
        out = self.block_mms[b]
        if secs is not None:
            out = [e for e in out if e[0] in secs]
        return out

    def shard_x(self, x):
        out = []
        for c in range(self.n_cores):
            xc = np.zeros((self.npp, x.shape[1]), dtype=np.float32)
            xc[: self.npr] = x[c * self.npr : (c + 1) * self.npr]
            out.append(xc)
        return out

    def unshard_out(self, outs):
        return np.concatenate([o[: self.npr] for o in outs], axis=0)




@dataclass
class Cfg:
    n_cores: int
    in_dim: int = 256
    hid: int = 128
    n_layers: int = 4
    out_dim: int = 7
    lamda: float = 0.5
    eps: float = 1e-5
    debug: bool = False
    gdt: object = None  # g/gather/one-hot dtype (mybir dt); None -> float32


def build(cfg: Cfg, P):
    """P: Prep instance (tile structure shared by all cores)."""
    nc = bacc.Bacc("TRN2", target_bir_lowering=False, debug=False, num_swdge_queues=4)
    NB = P.n_blocks
    H = cfg.hid
    CH = P.chunk_tiles
    NT = P.n_tiles
    NM = P.n_mms
    GDT = cfg.gdt if cfg.gdt is not None else F32
    # per-section mcol ranges (mcols assigned section-major in Prep)
    mc_base = [min((mc for bb in range(NB) for ss, jj, mc in P.block_mms[bb]
                    if ss == s_), default=0) for s_ in range(P.n_sec)]
    NSEC = P.n_sec
    SSZ = P.sec_size
    L = cfg.n_layers
    thetas = [cfg.lamda / (i + 1) for i in range(L)]

    x_ext = nc.declare_dram_parameter("x", [P.npp, cfg.in_dim], F32, isOutput=False)
    idx_ext = nc.declare_dram_parameter("idx16", [128, NT * 8], I16, isOutput=False)
    doff_ext = nc.declare_dram_parameter("doff", [128, NM], F32, isOutput=False)
    dinv_ext = nc.declare_dram_parameter("dinv", [128, NB], F32, isOutput=False)
    consts_ext = nc.declare_dram_parameter("consts", [128, 256], F32, isOutput=False)
    Wp_ext = nc.declare_dram_parameter("W_proj", [cfg.in_dim, H], F32, isOutput=False)
    bp_ext = nc.declare_dram_parameter("b_proj", [H], F32, isOutput=False)
    gam_ext = nc.declare_dram_parameter("gamma", [H], F32, isOutput=False)
    bet_ext = nc.declare_dram_parameter("beta", [H], F32, isOutput=False)
    qw_ext = nc.declare_dram_parameter("q_w", [H, 1], F32, isOutput=False)
    qb_ext = nc.declare_dram_parameter("q_b", [1], F32, isOutput=False)
    cw_ext = nc.declare_dram_parameter("conv_w", [L, H, H], F32, isOutput=False)
    clw_ext = nc.declare_dram_parameter("cls_w", [H, cfg.out_dim], F32, isOutput=False)
    clb_ext = nc.declare_dram_parameter("cls_b", [cfg.out_dim], F32, isOutput=False)
    out_ext = nc.declare_dram_parameter("out", [P.npp, cfg.out_dim], F32, isOutput=True)
    if cfg.debug:
        dbg_h0 = nc.declare_dram_parameter("dbg_h0", [NB * 128, H], F32, isOutput=True)
        dbg_s = nc.declare_dram_parameter("dbg_s", [128, NB], F32, isOutput=True)
        dbg_hs = [nc.declare_dram_parameter(f"dbg_h_{i}", [NB * 128, H], F32,
                                            isOutput=True) for i in range(L)]

    rg = [list(range(cfg.n_cores))]

    with tile.TileContext(nc) as tc, ExitStack() as ctx:
        sing = ctx.enter_context(tc.tile_pool(name="sing", bufs=1))
        xio = ctx.enter_context(tc.tile_pool(name="xio", bufs=2))
        gat_p = ctx.enter_context(tc.tile_pool(name="gat", bufs=7))
        m_p = ctx.enter_context(tc.tile_pool(name="m_p", bufs=4))
        work = ctx.enter_context(tc.tile_pool(name="work", bufs=3))
        gout_p = ctx.enter_context(tc.tile_pool(name="gout", bufs=3))
        ps_spmm = ctx.enter_context(tc.tile_pool(name="ps_spmm", bufs=3, space="PSUM"))
        ps_tr = ctx.enter_context(tc.tile_pool(name="ps_tr", bufs=2, space="PSUM"))
        ps_mm = ctx.enter_context(tc.tile_pool(name="ps_mm", bufs=2, space="PSUM"))
        ps_sm = ctx.enter_context(tc.tile_pool(name="ps_sm", bufs=1, space="PSUM"))
        dram = ctx.enter_context(tc.tile_pool(name="dram", bufs=1, space="DRAM"))

        HR = P.half_rows
        g_selfs = [dram.tile([P.npp, H], GDT, name=f"g_self_{i}") for i in range(L)]
        g_halves = [[dram.tile([cfg.n_cores * HR, H], GDT, addr_space="Shared",
                               name=f"g_h{hf}_{i}") for hf in range(2)]
                    for i in range(L)]

        def emit_ag(i, hf):
            nc.gpsimd.collective_compute(
                "AllGather", ALU.bypass,
                ins=[g_selfs[i][hf * HR : (hf + 1) * HR, :].opt()],
                outs=[g_halves[i][hf][:].opt()], replica_groups=rg)

        # ---- resident SBUF ----
        h0_sb = sing.tile([128, NB, H], F32)
        s_raw = sing.tile([128, NB], F32)
        dinv = sing.tile([128, NB], F32)
        idx_sb = sing.tile([128, NT * 8], I16)
        doff_sb = sing.tile([128, NM], F32)
        consts = sing.tile([128, 256], F32)
        iota_bf = sing.tile([128, 128], GDT)
        eps_sb = sing.tile([128, 1], F32)
        qbm1_sb = sing.tile([128, 1], F32)
        gam_rep = sing.tile([128, H], F32)
        bet_rep = sing.tile([128, H], F32)
        bp_rep = sing.tile([128, H], F32)
        clb_rep = sing.tile([128, cfg.out_dim], F32)
        qw_sb = sing.tile([128, 1], F32)
        wp_sb = sing.tile([128, 2, H], F32)
        wc_sb = sing.tile([128, L, H], F32)
        clw_sb = sing.tile([128, cfg.out_dim], F32)
        iota_f = consts[:, 0:128]
        ident = consts[:, 128:256]

        def bcast(dst, src_ap):
            rep = bass.AP(tensor=src_ap.tensor, offset=src_ap.offset,
                          ap=[[0, 128]] + src_ap.ap)
            nc.sync.dma_start(out=dst, in_=rep)

        # ---- constants / weights ----
        nc.sync.dma_start(out=consts[:], in_=consts_ext[:])
        nc.scalar.copy(out=iota_bf[:], in_=iota_f)
        nc.vector.memset(eps_sb[:], cfg.eps)
        bcast(gam_rep[:], gam_ext[:])
        bcast(bet_rep[:], bet_ext[:])
        bcast(bp_rep[:], bp_ext[:])
        bcast(clb_rep[:], clb_ext[:])
        bcast(qbm1_sb[:], qb_ext[:])
        nc.vector.tensor_scalar_add(out=qbm1_sb[:], in0=qbm1_sb[:], scalar1=-1.0)
        nc.sync.dma_start(out=wp_sb[:], in_=Wp_ext[:].rearrange("(c k) h -> k c h", k=128))
        nc.sync.dma_start(out=qw_sb[:], in_=qw_ext[:])
        nc.sync.dma_start(out=clw_sb[:], in_=clw_ext[:])
        nc.sync.dma_start(out=dinv[:], in_=dinv_ext[:])
        for i in range(L):
            wc_f = work.tile([128, H], F32, tag="wc_f")
            nc.sync.dma_start(out=wc_f[:], in_=cw_ext[i])
            th = thetas[i]
            nc.scalar.mul(out=wc_sb[:, i, :], in_=wc_f[:], mul=th / (1.0 - th))
        nc.sync.dma_start(out=idx_sb[:], in_=idx_ext[:])
        nc.sync.dma_start(out=doff_sb[:], in_=doff_ext[:])

        def transpose_f(src_ap, name):
            tp = ps_tr.tile([128, 128], F32, tag="tp")
            nc.tensor.transpose(out=tp[:], in_=src_ap, identity=ident)
            tb = work.tile([128, 128], F32, tag=name)
            nc.scalar.copy(out=tb[:], in_=tp[:])
            return tb

        u_all = sing.tile([128, NB], F32)

        def gate(h_ap, b):
            # store pre-activation; sigmoid applied batched once per phase so
            # the ACT table set never flips inside the block loop
            hT = transpose_f(h_ap, "hT")
            gp = ps_sm.tile([128, 1], F32, tag="sm")
            nc.tensor.matmul(out=gp[:], lhsT=hT[:], rhs=qw_sb[:], start=True, stop=True)
            nc.scalar.copy(out=u_all[:, b : b + 1], in_=gp[:])

        def gate_apply():
            nc.scalar.activation(out=s_raw[:], in_=u_all[:], func=AF.Sigmoid,
                                 bias=qbm1_sb[:], scale=1.0)

        def layernorm(src, dst):
            st = work.tile([128, 6], F32, tag="bnst")
            nc.vector.bn_stats(out=st[:], in_=src)
            mv = work.tile([128, 2], F32, tag="bnmv")
            nc.vector.bn_aggr(out=mv[:], in_=st[:])
            nc.scalar.activation(out=mv[:, 1:2], in_=mv[:, 1:2], func=AF.Sqrt,
                                 bias=eps_sb[:], scale=1.0)
            nc.vector.reciprocal(out=mv[:, 1:2], in_=mv[:, 1:2])
            nc.vector.tensor_scalar(out=dst, in0=src, scalar1=mv[:, 0:1],
                                    scalar2=mv[:, 1:2], op0=ALU.subtract, op1=ALU.mult)
            nc.vector.tensor_mul(out=dst, in0=dst, in1=gam_rep[:])
            nc.vector.tensor_add(out=dst, in0=dst, in1=bet_rep[:])

        # ---- projection + g0 + layer-0 gate ----
        for b in range(NB):
            xb = xio.tile([128, cfg.in_dim], F32, tag="xb")
            nc.sync.dma_start(out=xb[:], in_=x_ext[b * 128 : (b + 1) * 128, :])
            pp = ps_mm.tile([128, H], F32, tag="mm")
            for c2 in range(2):
                xT = transpose_f(xb[:, c2 * 128 : (c2 + 1) * 128], "xT")
                nc.tensor.matmul(out=pp[:], lhsT=xT[:], rhs=wp_sb[:, c2, :],
                                 start=(c2 == 0), stop=(c2 == 1))
            v = work.tile([128, H], F32, tag="v")
            nc.vector.tensor_add(out=v[:], in0=pp[:], in1=bp_rep[:])
            layernorm(v[:], h0_sb[:, b, :])
            gate(h0_sb[:, b, :], b)
            gb = gout_p.tile([128, H], GDT, tag="gb")
            nc.vector.tensor_scalar_mul(out=gb[:], in0=h0_sb[:, b, :],
                                        scalar1=dinv[:, b : b + 1])
            nc.sync.dma_start(out=g_selfs[0][b * 128 : (b + 1) * 128, :], in_=gb[:])
            if b == NB // 2 - 1:
                emit_ag(0, 0)
            elif b == NB - 1:
                emit_ag(0, 1)
        gate_apply()
        if cfg.debug:
            for b in range(NB):
                nc.sync.dma_start(out=dbg_h0[b * 128 : (b + 1) * 128, :],
                                  in_=h0_sb[:, b, :])
            nc.sync.dma_start(out=dbg_s[:], in_=s_raw[:])

        # ---- layers ----
        hs1_acc = sing.tile([128, NB, H], F32)   # s1m * (half-A spmm partial)
        s1m_all = sing.tile([128, NB], F32)
        ss_all = sing.tile([128, NB], F32)

        for i in range(L):
            th = thetas[i]
            last = i == L - 1

            ch_next = [0] * NSEC
            mt_next = [0] * NSEC
            gat_tiles = {}
            mt_tiles = {}
            qctr = [0]

            def chunk(s, j):
                cidx = j // CH
                while ch_next[s] <= cidx:
                    cc = ch_next[s]
                    gt = gat_p.tile([128, CH, H], GDT, tag="gat",
                                    name=f"gt_{i}_{s}_{cc}")
                    col0 = (int(P.sec_tile_base[s]) + cc * CH) * 8
                    src = g_halves[i][s // 2]
                    off = (s % 2) * SSZ
                    nc.gpsimd.dma_gather(
                        gt[:], src[off : off + SSZ, :],
                        idx_sb[:, col0 : col0 + CH * 8], CH * 128, CH * 128, H,
                        queue_num=qctr[0] % 4)
                    qctr[0] += 1
                    gat_tiles[(s, cc)] = gt
                    ch_next[s] += 1
                return gat_tiles[(s, cidx)]

            def mchunk(s, mc):
                ml = mc - mc_base[s]
                cidx = ml // CH
                while mt_next[s] <= cidx:
                    cc = mt_next[s]
                    m0 = mc_base[s] + cc * CH
                    mt = m_p.tile([128, CH, 128], GDT, tag="mt",
                                  name=f"mt_{i}_{s}_{cc}")
                    for c3 in range(CH):
                        if m0 + c3 < NM:
                            nc.vector.tensor_scalar(
                                out=mt[:, c3, :], in0=iota_bf[:],
                                scalar1=doff_sb[:, m0 + c3 : m0 + c3 + 1],
                                scalar2=None, op0=ALU.is_equal)
                    mt_tiles[(s, cc)] = mt
                    mt_next[s] += 1
                return mt_tiles[(s, cidx)]

            def spmm_half(b, secs, tag):
                bt = P.block_tiles(b, secs)
                ntot = len(bt)
                if ntot == 0:
                    return None
                hp = ps_spmm.tile([128, H], F32, tag="hp", name=f"hp_{i}_{b}_{tag}")
                for k, (s, j, mc) in enumerate(bt):
                    gt = chunk(s, j)
                    mt = mchunk(s, mc)
                    nc.tensor.matmul(out=hp[:], lhsT=mt[:, (mc - mc_base[s]) % CH, :],
                                     rhs=gt[:, j % CH, :],
                                     start=(k == 0), stop=(k == ntot - 1))
                return hp

            # gate coefficients batched: 3 DVE ops per layer instead of 3/block
            nc.vector.tensor_scalar(out=s1m_all[:], in0=s_raw[:],
                                    scalar1=-(1.0 - th), scalar2=(1.0 - th),
                                    op0=ALU.mult, op1=ALU.add)
            nc.vector.tensor_mul(out=s1m_all[:], in0=s1m_all[:], in1=dinv[:])
            nc.vector.tensor_scalar_mul(out=ss_all[:], in0=s_raw[:],
                                        scalar1=(1.0 - th))

            # pass A: half-0 sources (sections 0,1); stash s1m*partial in SBUF
            for b in range(NB):
                hpA = spmm_half(b, (0, 1), "hpA")
                if hpA is not None:
                    nc.scalar.mul(out=hs1_acc[:, b, :], in_=hpA[:],
                                  mul=s1m_all[:, b : b + 1])
                else:
                    nc.vector.memset(hs1_acc[:, b, :], 0.0)

            # pass B: half-1 sources; blend with stashed partial, dense chain
            for b in range(NB):
                hpB = spmm_half(b, (2, 3), "hpB")
                t2b = work.tile([128, H], F32, tag="t2b")
                nc.scalar.mul(out=t2b[:], in_=h0_sb[:, b, :], mul=ss_all[:, b : b + 1])
                sup = work.tile([128, H], F32, tag="sup")
                if hpB is not None:
                    t1b = work.tile([128, H], F32, tag="t1b")
                    nc.vector.scalar_tensor_tensor(
                        out=t1b[:], in0=hpB[:], scalar=s1m_all[:, b : b + 1],
                        in1=hs1_acc[:, b, :], op0=ALU.mult, op1=ALU.add)
                    nc.vector.tensor_add(out=sup[:], in0=t1b[:], in1=t2b[:])
                else:
                    nc.vector.tensor_add(out=sup[:], in0=hs1_acc[:, b, :], in1=t2b[:])

                supT = transpose_f(sup[:], "supT")
                cp = ps_mm.tile([128, H], F32, tag="mm")
                nc.tensor.matmul(out=cp[:], lhsT=supT[:], rhs=wc_sb[:, i, :],
                                 start=True, stop=True)
                z = work.tile([128, H], F32, tag="z")
                nc.vector.tensor_add(out=z[:], in0=cp[:], in1=sup[:])
                nc.scalar.activation(out=z[:], in_=z[:], func=AF.Relu)
                h = work.tile([128, H], F32, tag="h")
                layernorm(z[:], h[:])
                if cfg.debug:
                    nc.sync.dma_start(out=dbg_hs[i][b * 128 : (b + 1) * 128, :], in_=h[:])

                if not last:
                    gate(h[:], b)
                    gb = gout_p.tile([128, H], GDT, tag="gb")
                    nc.vector.tensor_scalar_mul(out=gb[:], in0=h[:],
                                                scalar1=dinv[:, b : b + 1])
                    nc.sync.dma_start(out=g_selfs[i + 1][b * 128 : (b + 1) * 128, :],
                                      in_=gb[:])
                else:
                    hT = transpose_f(h[:], "hT")
                    lp = ps_sm.tile([128, cfg.out_dim], F32, tag="sm")
                    nc.tensor.matmul(out=lp[:], lhsT=hT[:], rhs=clw_sb[:],
                                     start=True, stop=True)
                    ob = gout_p.tile([128, cfg.out_dim], F32, tag="ob")
                    nc.vector.tensor_add(out=ob[:], in0=lp[:], in1=clb_rep[:])
                    nc.sync.dma_start(out=out_ext[b * 128 : (b + 1) * 128, :], in_=ob[:])
            if not last:
                gate_apply()
                # AG triggers AFTER all of this layer's gathers are issued:
                # gpsimd stalling on the AG input-wait then starves nothing,
                # while the trigger still fires as soon as the half is written
                emit_ag(i + 1, 0)
                emit_ag(i + 1, 1)

    nc.compile()
    return nc


def make_consts():
    consts = np.zeros((128, 256), dtype=np.float32)
    consts[:, 0:128] = np.arange(128, dtype=np.float32)[None, :]
    consts[:, 128:256] = np.eye(128, dtype=np.float32)
    return consts


def make_runner(nc, n_cores):
    install_neuronx_cc_hook()
    partition_name = nc.partition_id_tensor.name if nc.partition_id_tensor else None
    in_names, out_names, out_avals, zero_outs = [], [], [], []
    for alloc in nc.m.functions[0].allocations:
        if not isinstance(alloc, mybir.MemoryLocationSet):
            continue
        name = alloc.memorylocations[0].name
        if alloc.kind == "ExternalInput":
            if name != partition_name:
                in_names.append(name)
        elif alloc.kind == "ExternalOutput":
            shape = tuple(alloc.tensor_shape)
            dtype = mybir.dt.np(alloc.dtype)
            out_names.append(name)
            out_avals.append(jax.core.ShapedArray(shape, dtype))
            zero_outs.append(np.zeros(shape, dtype))
    n_params = len(in_names)
    n_outs = len(out_avals)
    all_in_names = in_names + out_names + ([partition_name] if partition_name else [])
    donate = tuple(range(n_params, n_params + n_outs))

    def _body(*args):
        operands = list(args)
        if partition_name is not None:
            operands.append(partition_id_tensor())
        outs = _bass_exec_p.bind(
            *operands, out_avals=tuple(out_avals), in_names=tuple(all_in_names),
            out_names=tuple(out_names), lowering_input_output_aliases=(),
            sim_require_finite=True, sim_require_nnan=True, nc=nc)
        return tuple(outs)

    devices = jax.devices()[:n_cores]
    mesh = Mesh(np.asarray(devices), ("core",))
    in_specs = (PartitionSpec("core"),) * (n_params + n_outs)
    out_specs = (PartitionSpec("core"),) * n_outs
    sharded = jax.jit(
        shard_map(_body, mesh=mesh, in_specs=in_specs, out_specs=out_specs,
                  check_rep=False),
        donate_argnums=donate, keep_unused=True)

    def prepare_inputs(in_maps):
        """concat per-core inputs and device_put them; reusable across calls."""
        concat = [np.concatenate([np.asarray(in_maps[c][k]) for c in range(n_cores)], 0)
                  for k in in_names]
        sh = jax.sharding.NamedSharding(mesh, PartitionSpec("core"))
        return [jax.device_put(a, sh) for a in concat]

    def fresh_zeros():
        sh = jax.sharding.NamedSharding(mesh, PartitionSpec("core"))
        return [jax.device_put(np.zeros((n_cores * z.shape[0], *z.shape[1:]), z.dtype), sh)
                for z in zero_outs]

    def run(dev_inputs, zeros):
        out_arrs = sharded(*dev_inputs, *zeros)
        return out_arrs

    def split_outs(out_arrs):
        return [{name: np.asarray(out_arrs[i]).reshape(n_cores, *out_avals[i].shape)[c]
                 for i, name in enumerate(out_names)} for c in range(n_cores)]

    return run, prepare_inputs, fresh_zeros, split_outs, out_names


_CACHE = {}


def _get_runner(edge_key, edge_index):
    if edge_key in _CACHE:
        return _CACHE[edge_key]
    P = Prep(edge_index, 100000, 8, n_sec=4, chunk_tiles=8)
    cfg = Cfg(n_cores=8, n_layers=4, gdt=G_DTYPE)
    nc = build(cfg, P)
    r = make_runner(nc, 8)
    _CACHE[edge_key] = (P, r)
    return _CACHE[edge_key]


def make_in_maps(P, x, W_proj, b_proj, gamma, beta, q_w, q_b, conv_w, cls_w, cls_b):
    xs = P.shard_x(np.asarray(x, np.float32))
    consts = make_consts()
    in_maps = []
    for c in range(8):
        in_maps.append(dict(
            x=xs[c], idx16=P.idx16[c], doff=P.doff[c], dinv=P.dinv[c],
            consts=consts,
            W_proj=np.asarray(W_proj, np.float32),
            b_proj=np.asarray(b_proj, np.float32),
            gamma=np.asarray(gamma, np.float32),
            beta=np.asarray(beta, np.float32),
            q_w=np.asarray(q_w, np.float32).reshape(128, 1),
            q_b=np.asarray(q_b, np.float32).reshape(1),
            conv_w=np.asarray(conv_w, np.float32),
            cls_w=np.asarray(cls_w, np.float32),
            cls_b=np.asarray(cls_b, np.float32)))
    return in_maps


def kernel(x, edge_index, W_proj, b_proj, gamma, beta, q_w, q_b, conv_w, cls_w,
           cls_b):
    x = np.asarray(x, dtype=np.float32)
    edge_index = np.asarray(edge_index)
    key = (edge_index.shape, int(edge_index[0, 0]), int(edge_index[1, -1]),
           int(edge_index.sum() % (1 << 62)))
    P, (run, prep_in, fresh_zeros, split_outs, _) = _get_runner(key, edge_index)
    in_maps = make_in_maps(P, x, W_proj, b_proj, gamma, beta, q_w, q_b, conv_w,
                           cls_w, cls_b)
    dev_in = prep_in(in_maps)
    outs = run(dev_in, fresh_zeros())
    [o.block_until_ready() for o in outs]
    res = split_outs(outs)
    return P.unshard_out([res[c]["out"] for c in range(8)]).astype(np.float32)


# revision 3
# speedup vs baseline: 1.0918x; 1.0918x over previous
"""Self-contained Trainium2 Bass kernel for nn_ADGCNForDialog (ADGCN GNN). v2

kernel(**inputs) takes the FULL unsharded inputs (as produced by
setup_inputs()) and returns the FULL [100000, 7] float32 output.

Strategy (graph/data parallel across 8 NeuronCores):
  - nodes row-sharded 12500/core (padded to 12544); small weights replicated
  - edges bucketed by (dest 128-row block, source section); SPMD-shared tile
    structure (max bucket size over cores)
  - degrees/dinv computed on HOST (static per edge_index) and passed in
  - g = dinv*h exchanged in bf16 via two half-node AllGathers per layer
  - spmm = dma_gather (bf16, 4 SWDGE queues) + one-hot matmul accumulation in
    PSUM; one-hot tiles built bf16, batched 8-tiles-per-DVE-op via stride-0
    broadcast access patterns
  - dense AOR/DLR/LN chain per block in f32
"""

from contextlib import ExitStack
from dataclasses import dataclass

import numpy as np

import concourse.bass as bass
import concourse.tile as tile
from concourse import bacc, mybir

import jax
from jax.sharding import Mesh, PartitionSpec
from jax.experimental.shard_map import shard_map
from concourse.bass2jax import _bass_exec_p, install_neuronx_cc_hook, partition_id_tensor

F32 = mybir.dt.float32
G_DTYPE = None  # module-level override; None -> f32
BF16 = mybir.dt.bfloat16
I16 = mybir.dt.int16
AF = mybir.ActivationFunctionType
ALU = mybir.AluOpType


def ceil_to(x, m):
    return (x + m - 1) // m * m


class Prep:
    def __init__(self, edge_index, n_nodes, n_cores, n_sec=4, chunk_tiles=8):
        row = np.asarray(edge_index[0], dtype=np.int64)
        col = np.asarray(edge_index[1], dtype=np.int64)
        assert n_nodes % n_cores == 0
        npr = n_nodes // n_cores
        npp = ceil_to(npr, 128)
        nb = npp // 128
        NG = npp * n_cores
        assert NG % n_sec == 0
        sec_size = NG // n_sec
        assert sec_size < 32768
        self.n_nodes, self.n_cores = n_nodes, n_cores
        self.npr, self.npp, self.n_blocks = npr, npp, nb
        self.NG, self.n_sec, self.sec_size = NG, n_sec, sec_size
        self.chunk_tiles = chunk_tiles

        owner = row // npr
        ldest = row % npr
        half = npp // 2
        self.half_rows = half
        csrc = col // npr
        rsrc = col % npr
        hf = rsrc // half
        colp = hf * (n_cores * half) + csrc * half + (rsrc - hf * half)
        sec = colp // sec_size
        loc = (colp % sec_size).astype(np.int64)
        blk = ldest // 128

        counts = np.zeros((n_cores, nb, n_sec), dtype=np.int64)
        np.add.at(counts, (owner, blk, sec), 1)
        M_bs = counts.max(axis=0)                       # [nb, n_sec] padded bucket len
        self.M_bs = M_bs

        # bucket start offsets within each section stream
        B0 = np.zeros((nb, n_sec), dtype=np.int64)
        for s in range(n_sec):
            B0[1:, s] = np.cumsum(M_bs[:-1, s])
        self.B0 = B0
        sec_len = M_bs.sum(axis=0)                      # stream rows per section
        sec_ntiles = np.array([ceil_to(ceil_to(int(x), 128) // 128, chunk_tiles)
                               for x in sec_len])
        self.sec_ntiles = sec_ntiles
        sec_tile_base = np.zeros(n_sec + 1, dtype=np.int64)
        sec_tile_base[1:] = np.cumsum(sec_ntiles)
        self.sec_tile_base = sec_tile_base
        NT = int(sec_tile_base[-1])
        self.n_tiles = NT

        # matmul plan: per (b, s) the tiles its bucket overlaps; one doff
        # column per (b, s, tile) triple, in a fixed shared order
        self.block_mms = [[] for _ in range(nb)]        # b -> [(s, tile_local_j, mcol)]
        mcol = 0
        mm_key = {}
        for s in range(n_sec):
            for b in range(nb):
                if M_bs[b, s] == 0:
                    continue
                j_lo = int(B0[b, s]) // 128
                j_hi = int(B0[b, s] + M_bs[b, s] - 1) // 128
                for j in range(j_lo, j_hi + 1):
                    mm_key[(s, b, j)] = mcol
                    self.block_mms[b].append((s, j, mcol))
                    mcol += 1
        NM = mcol
        self.n_mms = NM

        # degrees -> dinv (host)
        deg = np.zeros((n_cores, npp), dtype=np.int64)
        np.add.at(deg, (owner, ldest), 1)
        degf = deg.astype(np.float64)
        degf[degf == 0] = 1.0
        dinv = (degf ** -0.5).astype(np.float32)
        self.dinv = np.ascontiguousarray(
            dinv.reshape(n_cores, nb, 128).transpose(0, 2, 1))

        # per-core metadata
        self.doff = np.full((n_cores, 128, NM), 200.0, dtype=np.float32)
        self.idx16 = np.zeros((n_cores, 128, NT * 8), dtype=np.int16)
        for c in range(n_cores):
            m = owner == c
            ld_c, lc_c, bl_c, se_c = ldest[m], loc[m], blk[m], sec[m]
            order = np.lexsort((ld_c, bl_c, se_c))
            ld_c, lc_c, bl_c, se_c = (a[order] for a in (ld_c, lc_c, bl_c, se_c))
            key = se_c * nb + bl_c
            first = np.r_[True, key[1:] != key[:-1]]
            grp_start = np.flatnonzero(first)
            pos = np.arange(len(key)) - np.repeat(
                grp_start, np.diff(np.r_[grp_start, len(key)]))
            i = B0[bl_c, se_c] + pos                     # stream position
            j = i // 128                                 # tile within section
            slot = i % 128
            mc = np.array([mm_key[(s_, b_, j_)]
                           for s_, b_, j_ in zip(se_c, bl_c, j)], dtype=np.int64)
            self.doff[c, slot, mc] = (ld_c - bl_c * 128).astype(np.float32)
            colno = sec_tile_base[se_c] * 8 + i // 16
            prow = i % 16
            for r in range(8):
                self.idx16[c, prow + 16 * r, colno] = lc_c.astype(np.int16)

    def block_tiles(self, b, secs=None):
        """[(s, j, mcol)] matmul list for block b, optionally filtered."""
        out = self.block_mms[b]
        if secs is not None:
            out = [e for e in out if e[0] in secs]
        return out

    def shard_x(self, x):
        out = []
        for c in range(self.n_cores):
            xc = np.zeros((self.npp, x.shape[1]), dtype=np.float32)
            xc[: self.npr] = x[c * self.npr : (c + 1) * self.npr]
            out.append(xc)
        return out

    def unshard_out(self, outs):
        return np.concatenate([o[: self.npr] for o in outs], axis=0)




@dataclass
class Cfg:
    n_cores: int
    in_dim: int = 256
    hid: int = 128
    n_layers: int = 4
    out_dim: int = 7
    lamda: float = 0.5
    eps: float = 1e-5
    debug: bool = False
    gdt: object = None  # g/gather/one-hot dtype (mybir dt); None -> float32


def build(cfg: Cfg, P):
    """P: Prep instance (tile structure shared by all cores)."""
    nc = bacc.Bacc("TRN2", target_bir_lowering=False, debug=False, num_swdge_queues=4)
    NB = P.n_blocks
    H = cfg.hid
    CH = P.chunk_tiles
    NT = P.n_tiles
    NM = P.n_mms
    GDT = cfg.gdt if cfg.gdt is not None else F32
    # per-section mcol ranges (mcols assigned section-major in Prep)
    mc_base = [min((mc for bb in range(NB) for ss, jj, mc in P.block_mms[bb]
                    if ss == s_), default=0) for s_ in range(P.n_sec)]
    NSEC = P.n_sec
    SSZ = P.sec_size
    L = cfg.n_layers
    thetas = [cfg.lamda / (i + 1) for i in range(L)]

    x_ext = nc.declare_dram_parameter("x", [P.npp, cfg.in_dim], F32, isOutput=False)
    idx_ext = nc.declare_dram_parameter("idx16", [128, NT * 8], I16, isOutput=False)
    doff_ext = nc.declare_dram_parameter("doff", [128, NM], F32, isOutput=False)
    dinv_ext = nc.declare_dram_parameter("dinv", [128, NB], F32, isOutput=False)
    consts_ext = nc.declare_dram_parameter("consts", [128, 256], F32, isOutput=False)
    Wp_ext = nc.declare_dram_parameter("W_proj", [cfg.in_dim, H], F32, isOutput=False)
    bp_ext = nc.declare_dram_parameter("b_proj", [H], F32, isOutput=False)
    gam_ext = nc.declare_dram_parameter("gamma", [H], F32, isOutput=False)
    bet_ext = nc.declare_dram_parameter("beta", [H], F32, isOutput=False)
    qw_ext = nc.declare_dram_parameter("q_w", [H, 1], F32, isOutput=False)
    qb_ext = nc.declare_dram_parameter("q_b", [1], F32, isOutput=False)
    cw_ext = nc.declare_dram_parameter("conv_w", [L, H, H], F32, isOutput=False)
    clw_ext = nc.declare_dram_parameter("cls_w", [H, cfg.out_dim], F32, isOutput=False)
    clb_ext = nc.declare_dram_parameter("cls_b", [cfg.out_dim], F32, isOutput=False)
    out_ext = nc.declare_dram_parameter("out", [P.npp, cfg.out_dim], F32, isOutput=True)
    if cfg.debug:
        dbg_h0 = nc.declare_dram_parameter("dbg_h0", [NB * 128, H], F32, isOutput=True)
        dbg_s = nc.declare_dram_parameter("dbg_s", [128, NB], F32, isOutput=True)
        dbg_hs = [nc.declare_dram_parameter(f"dbg_h_{i}", [NB * 128, H], F32,
                                            isOutput=True) for i in range(L)]

    rg = [list(range(cfg.n_cores))]

    with tile.TileContext(nc) as tc, ExitStack() as ctx:
        sing = ctx.enter_context(tc.tile_pool(name="sing", bufs=1))
        xio = ctx.enter_context(tc.tile_pool(name="xio", bufs=2))
        gat_p = ctx.enter_context(tc.tile_pool(name="gat", bufs=7))
        m_p = ctx.enter_context(tc.tile_pool(name="m_p", bufs=4))
        work = ctx.enter_context(tc.tile_pool(name="work", bufs=3))
        gout_p = ctx.enter_context(tc.tile_pool(name="gout", bufs=3))
        ps_spmm = ctx.enter_context(tc.tile_pool(name="ps_spmm", bufs=3, space="PSUM"))
        ps_tr = ctx.enter_context(tc.tile_pool(name="ps_tr", bufs=2, space="PSUM"))
        ps_mm = ctx.enter_context(tc.tile_pool(name="ps_mm", bufs=2, space="PSUM"))
        ps_sm = ctx.enter_context(tc.tile_pool(name="ps_sm", bufs=1, space="PSUM"))
        dram = ctx.enter_context(tc.tile_pool(name="dram", bufs=1, space="DRAM"))

        HR = P.half_rows
        g_selfs = [dram.tile([P.npp, H], GDT, name=f"g_self_{i}") for i in range(L)]
        g_halves = [[dram.tile([cfg.n_cores * HR, H], GDT, addr_space="Shared",
                               name=f"g_h{hf}_{i}") for hf in range(2)]
                    for i in range(L)]

        def emit_ag(i, hf):
            nc.gpsimd.collective_compute(
                "AllGather", ALU.bypass,
                ins=[g_selfs[i][hf * HR : (hf + 1) * HR, :].opt()],
                outs=[g_halves[i][hf][:].opt()], replica_groups=rg)

        # ---- resident SBUF ----
        h0_sb = sing.tile([128, NB, H], F32)
        s_raw = sing.tile([128, NB], F32)
        dinv = sing.tile([128, NB], F32)
        idx_sb = sing.tile([128, NT * 8], I16)
        doff_sb = sing.tile([128, NM], F32)
        consts = sing.tile([128, 256], F32)
        iota_bf = sing.tile([128, 128], GDT)
        eps_sb = sing.tile([128, 1], F32)
        qbm1_sb = sing.tile([128, 1], F32)
        gam_rep = sing.tile([128, H], F32)
        bet_rep = sing.tile([128, H], F32)
        bp_rep = sing.tile([128, H], F32)
        clb_rep = sing.tile([128, cfg.out_dim], F32)
        qw_sb = sing.tile([128, 1], F32)
        wp_sb = sing.tile([128, 2, H], F32)
        wc_sb = sing.tile([128, L, H], F32)
        clw_sb = sing.tile([128, cfg.out_dim], F32)
        iota_f = consts[:, 0:128]
        ident = consts[:, 128:256]

        def bcast(dst, src_ap):
            rep = bass.AP(tensor=src_ap.tensor, offset=src_ap.offset,
                          ap=[[0, 128]] + src_ap.ap)
            nc.sync.dma_start(out=dst, in_=rep)

        # ---- constants / weights ----
        nc.sync.dma_start(out=consts[:], in_=consts_ext[:])
        nc.scalar.copy(out=iota_bf[:], in_=iota_f)
        nc.vector.memset(eps_sb[:], cfg.eps)
        bcast(gam_rep[:], gam_ext[:])
        bcast(bet_rep[:], bet_ext[:])
        bcast(bp_rep[:], bp_ext[:])
        bcast(clb_rep[:], clb_ext[:])
        bcast(qbm1_sb[:], qb_ext[:])
        nc.vector.tensor_scalar_add(out=qbm1_sb[:], in0=qbm1_sb[:], scalar1=-1.0)
        nc.sync.dma_start(out=wp_sb[:], in_=Wp_ext[:].rearrange("(c k) h -> k c h", k=128))
        nc.sync.dma_start(out=qw_sb[:], in_=qw_ext[:])
        nc.sync.dma_start(out=clw_sb[:], in_=clw_ext[:])
        nc.sync.dma_start(out=dinv[:], in_=dinv_ext[:])
        for i in range(L):
            wc_f = work.tile([128, H], F32, tag="wc_f")
            nc.sync.dma_start(out=wc_f[:], in_=cw_ext[i])
            th = thetas[i]
            nc.scalar.mul(out=wc_sb[:, i, :], in_=wc_f[:], mul=th / (1.0 - th))
        nc.sync.dma_start(out=idx_sb[:], in_=idx_ext[:])
        nc.sync.dma_start(out=doff_sb[:], in_=doff_ext[:])

        def transpose_f(src_ap, name):
            tp = ps_tr.tile([128, 128], F32, tag="tp")
            nc.tensor.transpose(out=tp[:], in_=src_ap, identity=ident)
            tb = work.tile([128, 128], F32, tag=name)
            nc.scalar.copy(out=tb[:], in_=tp[:])
            return tb

        u_all = sing.tile([128, NB], F32)

        def gate(h_ap, b):
            # store pre-activation; sigmoid applied batched once per phase so
            # the ACT table set never flips inside the block loop
            hT = transpose_f(h_ap, "hT")
            gp = ps_sm.tile([128, 1], F32, tag="sm")
            nc.tensor.matmul(out=gp[:], lhsT=hT[:], rhs=qw_sb[:], start=True, stop=True)
            nc.scalar.copy(out=u_all[:, b : b + 1], in_=gp[:])

        def gate_apply():
            nc.scalar.activation(out=s_raw[:], in_=u_all[:], func=AF.Sigmoid,
                                 bias=qbm1_sb[:], scale=1.0)

        def layernorm(src, dst):
            st = work.tile([128, 6], F32, tag="bnst")
            nc.vector.bn_stats(out=st[:], in_=src)
            mv = work.tile([128, 2], F32, tag="bnmv")
            nc.vector.bn_aggr(out=mv[:], in_=st[:])
            nc.scalar.activation(out=mv[:, 1:2], in_=mv[:, 1:2], func=AF.Sqrt,
                                 bias=eps_sb[:], scale=1.0)
            nc.vector.reciprocal(out=mv[:, 1:2], in_=mv[:, 1:2])
            nc.vector.tensor_scalar(out=dst, in0=src, scalar1=mv[:, 0:1],
                                    scalar2=mv[:, 1:2], op0=ALU.subtract, op1=ALU.mult)
            nc.vector.tensor_mul(out=dst, in0=dst, in1=gam_rep[:])
            nc.vector.tensor_add(out=dst, in0=dst, in1=bet_rep[:])

        # ---- projection + g0 + layer-0 gate ----
        for b in range(NB):
            xb = xio.tile([128, cfg.in_dim], F32, tag="xb")
            nc.sync.dma_start(out=xb[:], in_=x_ext[b * 128 : (b + 1) * 128, :])
            pp = ps_mm.tile([128, H], F32, tag="mm")
            for c2 in range(2):
                xT = transpose_f(xb[:, c2 * 128 : (c2 + 1) * 128], "xT")
                nc.tensor.matmul(out=pp[:], lhsT=xT[:], rhs=wp_sb[:, c2, :],
                                 start=(c2 == 0), stop=(c2 == 1))
            v = work.tile([128, H], F32, tag="v")
            nc.vector.tensor_add(out=v[:], in0=pp[:], in1=bp_rep[:])
            layernorm(v[:], h0_sb[:, b, :])
            gate(h0_sb[:, b, :], b)
            gb = gout_p.tile([128, H], GDT, tag="gb")
            nc.vector.tensor_scalar_mul(out=gb[:], in0=h0_sb[:, b, :],
                                        scalar1=dinv[:, b : b + 1])
            nc.sync.dma_start(out=g_selfs[0][b * 128 : (b + 1) * 128, :], in_=gb[:])
            if b == NB // 2 - 1:
                emit_ag(0, 0)
            elif b == NB - 1:
                emit_ag(0, 1)
        gate_apply()
        if cfg.debug:
            for b in range(NB):
                nc.sync.dma_start(out=dbg_h0[b * 128 : (b + 1) * 128, :],
                                  in_=h0_sb[:, b, :])
            nc.sync.dma_start(out=dbg_s[:], in_=s_raw[:])

        # ---- layers ----
        hs1_acc = sing.tile([128, NB, H], F32)   # s1m * (half-A spmm partial)
        s1m_all = sing.tile([128, NB], F32)
        ss_all = sing.tile([128, NB], F32)

        for i in range(L):
            th = thetas[i]
            last = i == L - 1

            ch_next = [0] * NSEC
            mt_next = [0] * NSEC
            gat_tiles = {}
            mt_tiles = {}
            qctr = [0]

            def chunk(s, j):
                cidx = j // CH
                while ch_next[s] <= cidx:
                    cc = ch_next[s]
                    gt = gat_p.tile([128, CH, H], GDT, tag="gat",
                                    name=f"gt_{i}_{s}_{cc}")
                    col0 = (int(P.sec_tile_base[s]) + cc * CH) * 8
                    src = g_halves[i][s // 2]
                    off = (s % 2) * SSZ
                    nc.gpsimd.dma_gather(
                        gt[:], src[off : off + SSZ, :],
                        idx_sb[:, col0 : col0 + CH * 8], CH * 128, CH * 128, H,
                        queue_num=qctr[0] % 4)
                    qctr[0] += 1
                    gat_tiles[(s, cc)] = gt
                    ch_next[s] += 1
                return gat_tiles[(s, cidx)]

            def mchunk(s, mc):
                ml = mc - mc_base[s]
                cidx = ml // CH
                while mt_next[s] <= cidx:
                    cc = mt_next[s]
                    m0 = mc_base[s] + cc * CH
                    mt = m_p.tile([128, CH, 128], GDT, tag="mt",
                                  name=f"mt_{i}_{s}_{cc}")
                    nch = min(CH, NM - m0)
                    if nch == CH:
                        # batched: one DVE op builds all CH one-hots via
                        # stride-0 broadcast APs (verified identical to the
                        # per-column build)
                        io_ap = iota_bf[:]
                        d_ap = doff_sb[:, m0 : m0 + CH]
                        in0 = bass.AP(tensor=io_ap.tensor, offset=io_ap.offset,
                                      ap=[io_ap.ap[0], [0, CH], io_ap.ap[1]])
                        in1 = bass.AP(tensor=d_ap.tensor, offset=d_ap.offset,
                                      ap=[d_ap.ap[0], d_ap.ap[1], [0, 128]])
                        nc.vector.tensor_tensor(out=mt[:], in0=in0, in1=in1,
                                                op=ALU.is_equal)
                    else:
                        for c3 in range(nch):
                            nc.vector.tensor_scalar(
                                out=mt[:, c3, :], in0=iota_bf[:],
                                scalar1=doff_sb[:, m0 + c3 : m0 + c3 + 1],
                                scalar2=None, op0=ALU.is_equal)
                    mt_tiles[(s, cc)] = mt
                    mt_next[s] += 1
                return mt_tiles[(s, cidx)]

            def spmm_half(b, secs, tag):
                bt = P.block_tiles(b, secs)
                ntot = len(bt)
                if ntot == 0:
                    return None
                hp = ps_spmm.tile([128, H], F32, tag="hp", name=f"hp_{i}_{b}_{tag}")
                for k, (s, j, mc) in enumerate(bt):
                    gt = chunk(s, j)
                    mt = mchunk(s, mc)
                    nc.tensor.matmul(out=hp[:], lhsT=mt[:, (mc - mc_base[s]) % CH, :],
                                     rhs=gt[:, j % CH, :],
                                     start=(k == 0), stop=(k == ntot - 1))
                return hp

            # gate coefficients batched: 3 DVE ops per layer instead of 3/block
            nc.vector.tensor_scalar(out=s1m_all[:], in0=s_raw[:],
                                    scalar1=-(1.0 - th), scalar2=(1.0 - th),
                                    op0=ALU.mult, op1=ALU.add)
            nc.vector.tensor_mul(out=s1m_all[:], in0=s1m_all[:], in1=dinv[:])
            nc.vector.tensor_scalar_mul(out=ss_all[:], in0=s_raw[:],
                                        scalar1=(1.0 - th))

            # pass A: half-0 sources (sections 0,1); stash s1m*partial in SBUF
            for b in range(NB):
                hpA = spmm_half(b, (0, 1), "hpA")
                if hpA is not None:
                    nc.scalar.mul(out=hs1_acc[:, b, :], in_=hpA[:],
                                  mul=s1m_all[:, b : b + 1])
                else:
                    nc.vector.memset(hs1_acc[:, b, :], 0.0)

            # pass B: half-1 sources; blend with stashed partial, dense chain
            for b in range(NB):
                hpB = spmm_half(b, (2, 3), "hpB")
                t2b = work.tile([128, H], F32, tag="t2b")
                nc.scalar.mul(out=t2b[:], in_=h0_sb[:, b, :], mul=ss_all[:, b : b + 1])
                sup = work.tile([128, H], F32, tag="sup")
                if hpB is not None:
                    t1b = work.tile([128, H], F32, tag="t1b")
                    nc.vector.scalar_tensor_tensor(
                        out=t1b[:], in0=hpB[:], scalar=s1m_all[:, b : b + 1],
                        in1=hs1_acc[:, b, :], op0=ALU.mult, op1=ALU.add)
                    nc.vector.tensor_add(out=sup[:], in0=t1b[:], in1=t2b[:])
                else:
                    nc.vector.tensor_add(out=sup[:], in0=hs1_acc[:, b, :], in1=t2b[:])

                supT = transpose_f(sup[:], "supT")
                cp = ps_mm.tile([128, H], F32, tag="mm")
                nc.tensor.matmul(out=cp[:], lhsT=supT[:], rhs=wc_sb[:, i, :],
                                 start=True, stop=True)
                z = work.tile([128, H], F32, tag="z")
                nc.vector.tensor_add(out=z[:], in0=cp[:], in1=sup[:])
                nc.scalar.activation(out=z[:], in_=z[:], func=AF.Relu)
                h = work.tile([128, H], F32, tag="h")
                layernorm(z[:], h[:])
                if cfg.debug:
                    nc.sync.dma_start(out=dbg_hs[i][b * 128 : (b + 1) * 128, :], in_=h[:])

                if not last:
                    gate(h[:], b)
                    gb = gout_p.tile([128, H], GDT, tag="gb")
                    nc.vector.tensor_scalar_mul(out=gb[:], in0=h[:],
                                                scalar1=dinv[:, b : b + 1])
                    nc.sync.dma_start(out=g_selfs[i + 1][b * 128 : (b + 1) * 128, :],
                                      in_=gb[:])
                else:
                    hT = transpose_f(h[:], "hT")
                    lp = ps_sm.tile([128, cfg.out_dim], F32, tag="sm")
                    nc.tensor.matmul(out=lp[:], lhsT=hT[:], rhs=clw_sb[:],
                                     start=True, stop=True)
                    ob = gout_p.tile([128, cfg.out_dim], F32, tag="ob")
                    nc.vector.tensor_add(out=ob[:], in0=lp[:], in1=clb_rep[:])
                    nc.sync.dma_start(out=out_ext[b * 128 : (b + 1) * 128, :], in_=ob[:])
            if not last:
                gate_apply()
                # AG triggers AFTER all of this layer's gathers are issued:
                # gpsimd stalling on the AG input-wait then starves nothing,
                # while the trigger still fires as soon as the half is written
                emit_ag(i + 1, 0)
                emit_ag(i + 1, 1)

    nc.compile()
    return nc


def make_consts():
    consts = np.zeros((128, 256), dtype=np.float32)
    consts[:, 0:128] = np.arange(128, dtype=np.float32)[None, :]
    consts[:, 128:256] = np.eye(128, dtype=np.float32)
    return consts


def make_runner(nc, n_cores):
    install_neuronx_cc_hook()
    partition_name = nc.partition_id_tensor.name if nc.partition_id_tensor else None
    in_names, out_names, out_avals, zero_outs = [], [], [], []
    for alloc in nc.m.functions[0].allocations:
        if not isinstance(alloc, mybir.MemoryLocationSet):
            continue
        name = alloc.memorylocations[0].name
        if alloc.kind == "ExternalInput":
            if name != partition_name:
                in_names.append(name)
        elif alloc.kind == "ExternalOutput":
            shape = tuple(alloc.tensor_shape)
            dtype = mybir.dt.np(alloc.dtype)
            out_names.append(name)
            out_avals.append(jax.core.ShapedArray(shape, dtype))
            zero_outs.append(np.zeros(shape, dtype))
    n_params = len(in_names)
    n_outs = len(out_avals)
    all_in_names = in_names + out_names + ([partition_name] if partition_name else [])
    donate = tuple(range(n_params, n_params + n_outs))

    def _body(*args):
        operands = list(args)
        if partition_name is not None:
            operands.append(partition_id_tensor())
        outs = _bass_exec_p.bind(
            *operands, out_avals=tuple(out_avals), in_names=tuple(all_in_names),
            out_names=tuple(out_names), lowering_input_output_aliases=(),
            sim_require_finite=True, sim_require_nnan=True, nc=nc)
        return tuple(outs)

    devices = jax.devices()[:n_cores]
    mesh = Mesh(np.asarray(devices), ("core",))
    in_specs = (PartitionSpec("core"),) * (n_params + n_outs)
    out_specs = (PartitionSpec("core"),) * n_outs
    sharded = jax.jit(
        shard_map(_body, mesh=mesh, in_specs=in_specs, out_specs=out_specs,
                  check_rep=False),
        donate_argnums=donate, keep_unused=True)

    def prepare_inputs(in_maps):
        """concat per-core inputs and device_put them; reusable across calls."""
        concat = [np.concatenate([np.asarray(in_maps[c][k]) for c in range(n_cores)], 0)
                  for k in in_names]
        sh = jax.sharding.NamedSharding(mesh, PartitionSpec("core"))
        return [jax.device_put(a, sh) for a in concat]

    def fresh_zeros():
        sh = jax.sharding.NamedSharding(mesh, PartitionSpec("core"))
        return [jax.device_put(np.zeros((n_cores * z.shape[0], *z.shape[1:]), z.dtype), sh)
                for z in zero_outs]

    def run(dev_inputs, zeros):
        out_arrs = sharded(*dev_inputs, *zeros)
        return out_arrs

    def split_outs(out_arrs):
        return [{name: np.asarray(out_arrs[i]).reshape(n_cores, *out_avals[i].shape)[c]
                 for i, name in enumerate(out_names)} for c in range(n_cores)]

    return run, prepare_inputs, fresh_zeros, split_outs, out_names


_CACHE = {}


def _get_runner(edge_key, edge_index):
    if edge_key in _CACHE:
        return _CACHE[edge_key]
    P = Prep(edge_index, 100000, 8, n_sec=4, chunk_tiles=8)
    cfg = Cfg(n_cores=8, n_layers=4, gdt=G_DTYPE)
    nc = build(cfg, P)
    r = make_runner(nc, 8)
    _CACHE[edge_key] = (P, r)
    return _CACHE[edge_key]


def make_in_maps(P, x, W_proj, b_proj, gamma, beta, q_w, q_b, conv_w, cls_w, cls_b):
    xs = P.shard_x(np.asarray(x, np.float32))
    consts = make_consts()
    in_maps = []
    for c in range(8):
        in_maps.append(dict(
            x=xs[c], idx16=P.idx16[c], doff=P.doff[c], dinv=P.dinv[c],
            consts=consts,
            W_proj=np.asarray(W_proj, np.float32),
            b_proj=np.asarray(b_proj, np.float32),
            gamma=np.asarray(gamma, np.float32),
            beta=np.asarray(beta, np.float32),
            q_w=np.asarray(q_w, np.float32).reshape(128, 1),
            q_b=np.asarray(q_b, np.float32).reshape(1),
            conv_w=np.asarray(conv_w, np.float32),
            cls_w=np.asarray(cls_w, np.float32),
            cls_b=np.asarray(cls_b, np.float32)))
    return in_maps


def kernel(x, edge_index, W_proj, b_proj, gamma, beta, q_w, q_b, conv_w, cls_w,
           cls_b):
    x = np.asarray(x, dtype=np.float32)
    edge_index = np.asarray(edge_index)
    key = (edge_index.shape, int(edge_index[0, 0]), int(edge_index[1, -1]),
           int(edge_index.sum() % (1 << 62)))
    P, (run, prep_in, fresh_zeros, split_outs, _) = _get_runner(key, edge_index)
    in_maps = make_in_maps(P, x, W_proj, b_proj, gamma, beta, q_w, q_b, conv_w,
                           cls_w, cls_b)
    dev_in = prep_in(in_maps)
    outs = run(dev_in, fresh_zeros())
    [o.block_until_ready() for o in outs]
    res = split_outs(outs)
    return P.unshard_out([res[c]["out"] for c in range(8)]).astype(np.float32)
